# revision 8
# baseline (speedup 1.0000x reference)
"""DiffusionGraphConv Trainium2 kernel (8-core SPMD, data-parallel over batch).

Math refactoring (halves the big-matmul FLOPs vs the reference order):
  reference: out[b,n,o] = sum_{f,m} mats_m[n,f,b] * W[f*5+m, o]
  with mats = [x0, s0 x0, 2 s0^2 x0 - x0, s1 x0, 2 s1^2 x0 - x0].
  Projection (width F=128 -> O=64) commutes with the node-space diffusion, so:
    u_m = proj(x0, W_m)                       # [N, O, B] each, cheap
    out = (u0 - u2 - u4) + s0 (u1 + 2 s0 u2) + s1 (u3 + 2 s1 u4)
  Device computes c0 = u1 + s0 @ u2s, c1 = u3 + s1 @ u4s (u*s pre-scaled 2x),
  then out = v0 + s0 @ c0 + s1 @ c1 with the v0 term injected directly into
  the final PSUM accumulation group (stationary x0, moving 4096*Wv0), so no
  separate v0 staging pass or SBUF copy exists.

Pipeline per core (all supports RESIDENT in SBUF; loaded once per round):
  phase1: 64 matmuls [128f,128n]^T @ [128f,256] -> fp8 U slots (u1,u2s,u3,u4s)
          drained two-batches-per-bank (one big strided copy per psum bank,
          alternating DVE/ACT).
  hops:   c0/c1 via 8 fp8-DoubleRow matmuls per node tile (k=256 each),
          drain = DVE tensor_add into the u1/u3 slot in place.
  final:  per node tile: 8 tiny v0 matmuls seed the bank (start=True on the
          first: whole-bank pending-zero covers the rest), 16 DoubleRow
          matmuls accumulate 4096*(s0@c0+s1@c1), ACT scales psum by 1/4096
          straight to the output staging tile, DMA out.

Layouts (host-prepared, all "SBUF images"):
  x0t  [128 f, 16t*8b*128j] bf16: x0t[f, (t*8+b)*128+j] = cat(inputs,state)[b, t*128+j, f]
  wcat [128 f, 5*64]        bf16: [16*W1 | 2*W2/16 | 16*W3 | 2*W4/16 | 4096*Wv0]
  s*t  [16 t, 128 p, 2048]  fp8:  s*t[t, p, kt*128+j] = 256*s[t*128+j, kt*128+p]
       (strip t = transposed rows of s for output-node tile t, k-major)
  out  [2048 n, 8b*64o]     f32

Env quirks handled here: walrus accepts <=1 sync-wait per instruction
(_legalize_waits hoists extras onto EventSemaphore carriers; simulators need
legalize=False); repeat=N re-runs the idempotent pipeline for wall-clock
differencing since this axon terminal has no NTFF profiling.
"""

import sys

if "/opt/trn_rl_repo" not in sys.path:
    sys.path.insert(0, "/opt/trn_rl_repo")

import numpy as np
import ml_dtypes

import concourse.bass as bass
import concourse.mybir as mybir
from concourse.tile import TileContext
from concourse.bass_utils import run_bass_kernel_spmd

BF16 = mybir.dt.bfloat16
FP8 = mybir.dt.float8e4
NPFP8 = ml_dtypes.float8_e4m3
SCALE = 256.0
F32 = mybir.dt.float32
NPBF16 = ml_dtypes.bfloat16

N = 2048          # graph nodes
F = 128           # input_size (64 input + 64 hidden)
B = 64            # global batch
NCORES = 8
BS = B // NCORES  # 8 batches per core
O = 64            # output features
NT = N // 128     # 16 node tiles
M5 = 5            # diffusion matrices


def _legalize_waits(nc, max_waits=1):
    """Walrus in this env encodes at most one sync-wait per instruction.

    Tile's sem assignment can emit 2-3 waits on one instruction; hoist the
    excess onto standalone EventSemaphore carriers (same engine, inserted
    just before), which the sequencer executes in order — semantics are
    identical, encoding is legal."""
    f = nc.m.functions[0]
    for blk in f.blocks:
        new_insts = []
        changed = False
        for inst in blk.instructions:
            si = inst.sync_info
            waits = list(si.on_wait) if si is not None else []
            if len(waits) > max_waits:
                for i, w in enumerate(waits[:-max_waits]):
                    ev = mybir.InstEventSemaphore(
                        name=f"{inst.name}-wsplit{i}",
                        engine=inst.engine,
                        ins=[],
                        outs=[],
                        sync_info=mybir.SyncInfo(on_wait=[w], on_update=[]),
                    )
                    new_insts.append(ev)
                inst.sync_info = mybir.SyncInfo(
                    on_wait=waits[-max_waits:], on_update=list(si.on_update)
                )
                changed = True
            new_insts.append(inst)
        if changed:
            blk.instructions = new_insts
    return nc


def build_bass(n=N, bs=BS, o=O, legalize=True, repeat=1):
    """Build the per-core SPMD Bass program."""
    nt = n // 128
    nc = bass.Bass()
    x0t = nc.dram_tensor("x0t", [F, bs * n], BF16, kind="ExternalInput")
    wcat = nc.dram_tensor("wcat", [F, M5 * o], BF16, kind="ExternalInput")
    s0t = nc.dram_tensor("s0t", [nt, 128, n], FP8, kind="ExternalInput")
    s1t = nc.dram_tensor("s1t", [nt, 128, n], FP8, kind="ExternalInput")
    out = nc.dram_tensor("out", [n, bs * o], BF16, kind="ExternalOutput")

    obs = bs * o        # 512: width of diffusion operands
    with TileContext(nc) as tc:
        with (
            tc.tile_pool(name="persist", bufs=1) as persist,
            tc.tile_pool(name="vout", bufs=4) as voutp,
            tc.tile_pool(name="pproj", bufs=2, space="PSUM") as pproj,
            tc.tile_pool(name="pacc", bufs=3, space="PSUM") as pacc,
        ):
            w_sb = persist.tile([F, M5 * o], BF16, name="w_sb")
            x0_sb = persist.tile([F, bs * n], BF16, name="x0_sb")
            # Both supports stay resident (fp8 strip images, 32KB/partition
            # each); every hop/final matmul reads them from SBUF.
            s_sb = [
                persist.tile([128, nt * n], FP8, name=f"s{i}_sb", tag=f"s{i}_sb")
                for i in range(2)
            ]
            # U[tp]: [128, 4*2*obs] fp8, slots mi: 1=u1*16->c0*16, 2=u2s/16,
            # 3=u3*16->c1*16, 4=u4s/16; free = (mi-1)*2*obs + kt2*obs + b*64+o.
            U = [
                persist.tile([128, 4 * 2 * obs], FP8, name=f"u{tp}", tag=f"u{tp}")
                for tp in range(nt // 2)
            ]

            def upair(tp, mi):
                """[128, 2, obs] DoubleRow moving view: k-tile pair of slot mi."""
                return U[tp].rearrange("p (mi4 kt2 c) -> p mi4 kt2 c", mi4=4, kt2=2)[
                    :, mi - 1, :, :
                ]

            def uslot_w(t, mi):
                """[128, obs] contiguous write view of slot mi for node-tile t."""
                base = (mi - 1) * 2 * obs + (t % 2) * obs
                return U[t // 2][:, base:base + obs]

            def sstrip(si, t, ktp):
                """[128, 2, 128] DoubleRow stationary: sT block (k-pair, tile t)."""
                lo = t * n + ktp * 256
                return s_sb[si][:, lo:lo + 256].rearrange(
                    "p (kt2 j) -> p kt2 j", kt2=2
                )

            for _rep in range(repeat):
                # ---- input DMAs (sync queue, in need-order)
                nc.sync.dma_start(out=w_sb[:, :], in_=wcat[:, :])
                pos = 0
                for c in (1, 3, 4, 4, 4):   # x0 chunks: small first chunk so
                    lo, hi = pos * bs * 128, (pos + c) * bs * 128  # t=0 starts early
                    nc.sync.dma_start(out=x0_sb[:, lo:hi], in_=x0t[:, lo:hi])
                    pos += c
                for si, sd in enumerate((s0t, s1t)):
                    for t in range(nt):
                        nc.sync.dma_start(
                            out=s_sb[si][:, t * n:(t + 1) * n], in_=sd[t]
                        )

                # ---- Phase 1: project x0 into fp8 slots u1,u2s,u3,u4s.
                # Four batches share one 2-bank psum tile (free 4x256, each
                # matmul's range stays inside one bank); each tile is drained
                # by ONE strided copy, alternating DVE/ACT.
                cpy = 0
                for t in range(nt):
                    for bq in range(bs // 4):
                        ps = pproj.tile([128, 1024], F32, name="ps_proj", tag="proj")
                        for i in range(4):
                            b = bq * 4 + i
                            nc.tensor.matmul(
                                ps[:, i * 256:(i + 1) * 256],
                                lhsT=x0_sb[:, (t * bs + b) * 128:(t * bs + b + 1) * 128],
                                rhs=w_sb[:, 0:4 * o],
                                start=(i % 2 == 0),
                                stop=(i % 2 == 1),
                            )
                        # src (i4, mi4, o) -> dst U[t//2][mi4, t%2, bq*4+i4, o]
                        dst = U[t // 2].rearrange(
                            "p (mi4 kt2 b8 c) -> p mi4 kt2 b8 c", mi4=4, kt2=2, b8=bs
                        )[:, :, t % 2, bq * 4:(bq + 1) * 4, :]
                        src = ps.rearrange("p (i4 mi4 c) -> p mi4 i4 c", i4=4, mi4=4)
                        if cpy % 2 == 0:
                            nc.vector.tensor_copy(out=dst, in_=src)
                        else:
                            nc.scalar.copy(out=dst, in_=src)
                        cpy += 1

                # ---- Hops: c0 = u1 + s0 @ u2s ; c1 = u3 + s1 @ u4s.
                # psum = (256*s)@(u*s/16) = 16*(s @ u*s); slot holds 16*u ->
                # plain add keeps c at 16x scale (fp8-safe).
                for si, src_slot, dst_slot in ((0, 2, 1), (1, 4, 3)):
                    for t in range(nt):
                        ps = pacc.tile([128, obs], F32, name="ps_acc", tag="acc")
                        for ktp in range(nt // 2):
                            nc.tensor.matmul(
                                ps[:, :],
                                lhsT=sstrip(si, t, ktp),
                                rhs=upair(ktp, src_slot),
                                start=(ktp == 0),
                                stop=(ktp == nt // 2 - 1),
                                perf_mode=mybir.MatmulPerfMode.DoubleRow,
                            )
                        d = uslot_w(t, dst_slot)
                        nc.vector.tensor_add(d, d, ps[:, :])

                # ---- Final: psum = 4096*v0 + (256*s0)@(16*c0) + (256*s1)@(16*c1)
                #            = 4096 * out_tile ; ACT scales 1/4096 -> DMA out.
                # v0 is injected by 8 per-batch matmuls (stationary x0, moving
                # 4096*Wv0); the first carries start=True, whose whole-bank
                # pending-zero covers the other 7 disjoint ranges.
                for t in range(nt):
                    ps = pacc.tile([128, obs], F32, name="ps_fin", tag="acc")
                    for b in range(bs):
                        nc.tensor.matmul(
                            ps[:, b * o:(b + 1) * o],
                            lhsT=x0_sb[:, (t * bs + b) * 128:(t * bs + b + 1) * 128],
                            rhs=w_sb[:, 4 * o:5 * o],
                            start=(b == 0),
                            stop=False,
                        )
                    for g, sl in enumerate((1, 3)):
                        for ktp in range(nt // 2):
                            nc.tensor.matmul(
                                ps[:, :],
                                lhsT=sstrip(g, t, ktp),
                                rhs=upair(ktp, sl),
                                start=False,
                                stop=(g == 1 and ktp == nt // 2 - 1),
                                perf_mode=mybir.MatmulPerfMode.DoubleRow,
                            )
                    vo = voutp.tile([128, obs], BF16, name="vo", tag="vo")
                    nc.scalar.mul(out=vo[:, :], in_=ps[:, :], mul=1.0 / (SCALE * 16.0))
                    nc.sync.dma_start(
                        out=out[t * 128:(t + 1) * 128, :], in_=vo[:, :]
                    )
    return _legalize_waits(nc) if legalize else nc


_NC_CACHE = {}


def _get_nc():
    if "nc" not in _NC_CACHE:
        _NC_CACHE["nc"] = build_bass()
    return _NC_CACHE["nc"]


def make_inputs(support0, support1, inputs, state, weight):
    """Host-side layout prep -> per-core in_maps (shared replicated arrays)."""
    xs = np.concatenate(
        [
            np.asarray(inputs, np.float32).reshape(B, N, F // 2),
            np.asarray(state, np.float32).reshape(B, N, F // 2),
        ],
        axis=2,
    )  # [B, N, F]

    w = np.asarray(weight, np.float32).reshape(F, M5, O)
    wv0 = w[:, 0] - w[:, 2] - w[:, 4]
    wcat = np.concatenate(
        [16.0 * w[:, 1], 2.0 * w[:, 2] / 16.0,
         16.0 * w[:, 3], 2.0 * w[:, 4] / 16.0,
         SCALE * 16.0 * wv0], axis=1
    ).astype(NPBF16)  # [128, 320]; hop slots scaled so fp8 adds stay in-range

    def strip_img(s):
        # fp8 DoubleRow pair layout: [t, p, ktp*256 + kt2*128 + j]
        #   = fp8(SCALE * s[t*128+j, (ktp*2+kt2)*128 + p])
        r = (SCALE * np.asarray(s, np.float32)).astype(NPFP8)
        r = r.reshape(NT, 128, NT, 128).transpose(0, 3, 2, 1)  # [t, p, kt, j]
        return np.ascontiguousarray(r.reshape(NT, 128, N))

    s0i, s1i = strip_img(support0), strip_img(support1)

    in_maps = []
    for c in range(NCORES):
        shard = xs[c * BS:(c + 1) * BS]                # [8b, N, F]
        # t-major SBUF image: x0t[f, t*BS*128 + b*128 + j] = shard[b, t*128+j, f]
        x0t = np.ascontiguousarray(
            shard.reshape(BS, NT, 128, F).transpose(3, 1, 0, 2).reshape(F, BS * N)
        ).astype(NPBF16)
        in_maps.append({"x0t": x0t, "wcat": wcat, "s0t": s0i, "s1t": s1i})
    return in_maps


def postprocess(results, biases):
    full = np.empty((B, N, O), np.float32)
    for c, r in enumerate(results):
        full[c * BS:(c + 1) * BS] = (
            np.asarray(r["out"], dtype=np.float32).reshape(N, BS, O).transpose(1, 0, 2)
        )
    full += np.asarray(biases, np.float32)[None, None, :]
    return full.reshape(B, N * O)


def kernel(support0, support1, inputs, state, weight, biases, output_size=None,
           **run_kwargs):
    nc = _get_nc()
    in_maps = make_inputs(support0, support1, inputs, state, weight)
    res = run_bass_kernel_spmd(nc, in_maps, core_ids=list(range(NCORES)),
                               **run_kwargs)
    out = postprocess(res.results, biases)
    if run_kwargs.get("trace"):
        return out, res
    return out


# revision 20
# speedup vs baseline: 1.1499x; 1.1499x over previous
"""DiffusionGraphConv Trainium2 kernel (8-core SPMD, data-parallel over batch).

Math refactoring (halves the big-matmul FLOPs vs the reference order):
  reference: out[b,n,o] = sum_{f,m} mats_m[n,f,b] * W[f*5+m, o]
  with mats = [x0, s0 x0, 2 s0^2 x0 - x0, s1 x0, 2 s1^2 x0 - x0].
  Projection (width F=128 -> O=64) commutes with the node-space diffusion, so:
    u_m = proj(x0, W_m)                       # [N, O, B] each, cheap
    out = (u0 - u2 - u4) + s0 (u1 + 2 s0 u2) + s1 (u3 + 2 s1 u4)
  Device computes c0 = u1 + s0 @ u2s, c1 = u3 + s1 @ u4s (u*s pre-scaled 2x),
  then out = v0 + s0 @ c0 + s1 @ c1 with the v0 term injected directly into
  the final PSUM accumulation group (stationary x0, moving 4096*Wv0), so no
  separate v0 staging pass or SBUF copy exists.

Pipeline per core (all supports RESIDENT in SBUF; loaded once per round):
  phase1: 64 matmuls [128f,128n]^T @ [128f,256] -> fp8 U slots (u1,u2s,u3,u4s)
          drained two-batches-per-bank (one big strided copy per psum bank,
          alternating DVE/ACT).
  hops:   c0/c1 via 8 fp8-DoubleRow matmuls per node tile (k=256 each),
          drain = DVE tensor_add into the u1/u3 slot in place.
  final:  per node tile: 8 tiny v0 matmuls seed the bank (start=True on the
          first: whole-bank pending-zero covers the rest), 16 DoubleRow
          matmuls accumulate 4096*(s0@c0+s1@c1), ACT scales psum by 1/4096
          straight to the output staging tile, DMA out.

Layouts (host-prepared, all "SBUF images"):
  x0t  [128 f, 16t*8b*128j] bf16: x0t[f, (t*8+b)*128+j] = cat(inputs,state)[b, t*128+j, f]
  wcat [128 f, 5*64]        bf16: [16*W1 | 2*W2/16 | 16*W3 | 2*W4/16 | 4096*Wv0]
  s*t  [16 t, 128 p, 2048]  fp8:  s*t[t, p, kt*128+j] = 256*s[t*128+j, kt*128+p]
       (strip t = transposed rows of s for output-node tile t, k-major)
  out  [2048 n, 8b*64o]     f32

Env quirks handled here: walrus accepts <=1 sync-wait per instruction
(_legalize_waits hoists extras onto EventSemaphore carriers; simulators need
legalize=False); repeat=N re-runs the idempotent pipeline for wall-clock
differencing since this axon terminal has no NTFF profiling.
"""

import sys

if "/opt/trn_rl_repo" not in sys.path:
    sys.path.insert(0, "/opt/trn_rl_repo")

import numpy as np
import ml_dtypes

import concourse.bass as bass
import concourse.mybir as mybir
from concourse.tile import TileContext
from concourse.bass_utils import run_bass_kernel_spmd

BF16 = mybir.dt.bfloat16
FP8 = mybir.dt.float8e4
NPFP8 = ml_dtypes.float8_e4m3
SCALE = 256.0
F32 = mybir.dt.float32
NPBF16 = ml_dtypes.bfloat16

N = 2048          # graph nodes
F = 128           # input_size (64 input + 64 hidden)
B = 64            # global batch
NCORES = 8
BS = B // NCORES  # 8 batches per core
O = 64            # output features
NT = N // 128     # 16 node tiles
M5 = 5            # diffusion matrices


def _legalize_waits(nc, max_waits=1):
    """Walrus in this env encodes at most one sync-wait per instruction.

    Tile's sem assignment can emit 2-3 waits on one instruction; hoist the
    excess onto standalone EventSemaphore carriers (same engine, inserted
    just before), which the sequencer executes in order — semantics are
    identical, encoding is legal."""
    f = nc.m.functions[0]
    for blk in f.blocks:
        new_insts = []
        changed = False
        for inst in blk.instructions:
            si = inst.sync_info
            waits = list(si.on_wait) if si is not None else []
            if len(waits) > max_waits:
                for i, w in enumerate(waits[:-max_waits]):
                    ev = mybir.InstEventSemaphore(
                        name=f"{inst.name}-wsplit{i}",
                        engine=inst.engine,
                        ins=[],
                        outs=[],
                        sync_info=mybir.SyncInfo(on_wait=[w], on_update=[]),
                    )
                    new_insts.append(ev)
                inst.sync_info = mybir.SyncInfo(
                    on_wait=waits[-max_waits:], on_update=list(si.on_update)
                )
                changed = True
            new_insts.append(inst)
        if changed:
            blk.instructions = new_insts
    return nc


def build_bass(n=N, bs=BS, o=O, legalize=True, repeat=1):
    """Build the per-core SPMD Bass program."""
    nt = n // 128
    nc = bass.Bass()
    x0t = nc.dram_tensor("x0t", [F, bs * n], BF16, kind="ExternalInput")
    wcat = nc.dram_tensor("wcat", [F, M5 * o], BF16, kind="ExternalInput")
    s0t = nc.dram_tensor("s0t", [nt, 128, n], FP8, kind="ExternalInput")
    s1t = nc.dram_tensor("s1t", [nt, 128, n], FP8, kind="ExternalInput")
    out = nc.dram_tensor("out", [n, bs * o], BF16, kind="ExternalOutput")

    obs = bs * o        # 512: width of diffusion operands
    with TileContext(nc) as tc:
        with (
            tc.tile_pool(name="persist", bufs=1) as persist,
            tc.tile_pool(name="vout", bufs=4) as voutp,
            tc.tile_pool(name="ppool", bufs=4, space="PSUM") as ppool,
        ):
            w_sb = persist.tile([F, M5 * o], BF16, name="w_sb")
            # Scratch operand for PE keep-warm matmuls: the cost model's (and
            # silicon's) p-state ramp runs the PE clock 2-4x slower until ~3us
            # of continuous execution; zero-dependency matmuls on garbage data
            # during the DMA head mean phase 1 starts at full clock.
            scr_in = persist.tile([128, 512], FP8, name="scr_in")
            x0_sb = persist.tile([F, bs * n], BF16, name="x0_sb")
            # Both supports stay resident (fp8 strip images, 32KB/partition
            # each); every hop/final matmul reads them from SBUF.
            s_sb = [
                persist.tile([128, nt * n], FP8, name=f"s{i}_sb", tag=f"s{i}_sb")
                for i in range(2)
            ]
            # U[tp]: [128, 4*2*obs] fp8, slots mi: 1=u1*16->c0*16, 2=u2s/16,
            # 3=u3*16->c1*16, 4=u4s/16; free = (mi-1)*2*obs + kt2*obs + b*64+o.
            U = [
                persist.tile([128, 4 * 2 * obs], FP8, name=f"u{tp}", tag=f"u{tp}")
                for tp in range(nt // 2)
            ]

            def upair(tp, mi):
                """[128, 2, obs] DoubleRow moving view: k-tile pair of slot mi."""
                return U[tp].rearrange("p (mi4 kt2 c) -> p mi4 kt2 c", mi4=4, kt2=2)[
                    :, mi - 1, :, :
                ]

            def uslot_w(t, mi):
                """[128, obs] contiguous write view of slot mi for node-tile t."""
                base = (mi - 1) * 2 * obs + (t % 2) * obs
                return U[t // 2][:, base:base + obs]

            def sstrip(si, t, ktp):
                """[128, 2, 128] DoubleRow stationary: sT block (k-pair, tile t)."""
                lo = t * n + ktp * 256
                return s_sb[si][:, lo:lo + 256].rearrange(
                    "p (kt2 j) -> p kt2 j", kt2=2
                )

            for _rep in range(repeat):
                if _rep == 0:
                    # allocate + define the keep-warm operand (DVE is idle
                    # and faster to first-issue than Pool's Q7 launch)
                    nc.vector.memset(scr_in[:, :], 0.0)
                # ---- input DMAs (descriptor-gen and the transfer engines are
                # both single shared serial resources: order = need-order, and
                # per-tile x0 granularity keeps phase 1 fed without big-chunk
                # stalls).
                for t in range(nt):
                    lo, hi = t * bs * 128, (t + 1) * bs * 128
                    nc.sync.dma_start(out=x0_sb[:, lo:hi], in_=x0t[:, lo:hi])
                    if t == 0:
                        nc.sync.dma_start(out=w_sb[:, :], in_=wcat[:, :])
                for si, sd in enumerate((s0t, s1t)):
                    for t in range(nt):
                        nc.sync.dma_start(
                            out=s_sb[si][:, t * n:(t + 1) * n], in_=sd[t]
                        )

                # PE warm-up while the first DMAs land (~1.6us to ~4.4us);
                # runs through the normal psum pool rotation, slot is
                # recycled by phase 1's 4th tile.
                ps_w = ppool.tile([128, 1024], F32, name="ps_w", tag="pp")
                for _ in range(11):
                    nc.tensor.matmul(
                        ps_w[:, 0:256],
                        lhsT=scr_in[:, 0:128],
                        rhs=scr_in[:, 0:256],
                        start=True, stop=True, skip_group_check=True,
                    )

                # ---- Phase 1: project x0 into fp8 slots u1,u2s,u3,u4s.
                # Four batches share one 2-bank psum tile (free 4x256, each
                # matmul's range stays inside one bank); each tile is drained
                # by ONE strided copy, alternating DVE/ACT.
                cpy = 0
                for t in range(nt):
                    for bq in range(bs // 4):
                        ps = ppool.tile([128, 1024], F32, name="ps_proj", tag="pp")
                        for i in range(4):
                            b = bq * 4 + i
                            nc.tensor.matmul(
                                ps[:, i * 256:(i + 1) * 256],
                                lhsT=x0_sb[:, (t * bs + b) * 128:(t * bs + b + 1) * 128],
                                rhs=w_sb[:, 0:4 * o],
                                start=(i % 2 == 0),
                                stop=(i % 2 == 1),
                            )
                        # src (i4, mi4, o) -> dst U[t//2][mi4, t%2, bq*4+i4, o]
                        dst = U[t // 2].rearrange(
                            "p (mi4 kt2 b8 c) -> p mi4 kt2 b8 c", mi4=4, kt2=2, b8=bs
                        )[:, :, t % 2, bq * 4:(bq + 1) * 4, :]
                        src = ps.rearrange("p (i4 mi4 c) -> p mi4 i4 c", i4=4, mi4=4)
                        # time-balanced split: DVE copy ~1192ns, ACT ~1038ns
                        # -> DVE gets the smaller share of the 32 copies.
                        if int(cpy * 0.466) > int((cpy - 1) * 0.466):
                            nc.vector.tensor_copy(out=dst, in_=src)
                        else:
                            nc.scalar.copy(out=dst, in_=src)
                        cpy += 1

                # ---- Hops: c0 = u1 + s0 @ u2s ; c1 = u3 + s1 @ u4s.
                # psum = (256*s)@(u*s/16) = 16*(s @ u*s); slot holds 16*u ->
                # plain add keeps c at 16x scale (fp8-safe).
                for si, src_slot, dst_slot in ((0, 2, 1), (1, 4, 3)):
                    for t in range(nt):
                        ps = ppool.tile([128, 1024], F32, name="ps_acc", tag="pp")[:, 0:obs]
                        for ktp in range(nt // 2):
                            nc.tensor.matmul(
                                ps[:, :],
                                lhsT=sstrip(si, t, ktp),
                                rhs=upair(ktp, src_slot),
                                start=(ktp == 0),
                                stop=(ktp == nt // 2 - 1),
                                perf_mode=mybir.MatmulPerfMode.DoubleRow,
                            )
                        d = uslot_w(t, dst_slot)
                        nc.vector.tensor_add(d, d, ps[:, :])

                # ---- Final: psum = 4096*v0 + (256*s0)@(16*c0) + (256*s1)@(16*c1)
                #            = 4096 * out_tile ; ACT scales 1/4096 -> DMA out.
                # v0 is injected by 8 per-batch matmuls (stationary x0, moving
                # 4096*Wv0); the first carries start=True, whose whole-bank
                # pending-zero covers the other 7 disjoint ranges.
                for t in range(nt):
                    ps = ppool.tile([128, 1024], F32, name="ps_fin", tag="pp")[:, 0:obs]
                    for b in range(bs):
                        nc.tensor.matmul(
                            ps[:, b * o:(b + 1) * o],
                            lhsT=x0_sb[:, (t * bs + b) * 128:(t * bs + b + 1) * 128],
                            rhs=w_sb[:, 4 * o:5 * o],
                            start=(b == 0),
                            stop=False,
                        )
                    for g, sl in enumerate((1, 3)):
                        for ktp in range(nt // 2):
                            nc.tensor.matmul(
                                ps[:, :],
                                lhsT=sstrip(g, t, ktp),
                                rhs=upair(ktp, sl),
                                start=False,
                                stop=(g == 1 and ktp == nt // 2 - 1),
                                perf_mode=mybir.MatmulPerfMode.DoubleRow,
                            )
                    vo = voutp.tile([128, obs], BF16, name="vo", tag="vo")
                    nc.scalar.mul(out=vo[:, :], in_=ps[:, :],
                                  mul=1.0 / (SCALE * 16.0))
                    nc.sync.dma_start(
                        out=out[t * 128:(t + 1) * 128, :], in_=vo[:, :]
                    )
    return _legalize_waits(nc) if legalize else nc


_NC_CACHE = {}


def _get_nc():
    if "nc" not in _NC_CACHE:
        _NC_CACHE["nc"] = build_bass()
    return _NC_CACHE["nc"]


def make_inputs(support0, support1, inputs, state, weight):
    """Host-side layout prep -> per-core in_maps (shared replicated arrays)."""
    xs = np.concatenate(
        [
            np.asarray(inputs, np.float32).reshape(B, N, F // 2),
            np.asarray(state, np.float32).reshape(B, N, F // 2),
        ],
        axis=2,
    )  # [B, N, F]

    w = np.asarray(weight, np.float32).reshape(F, M5, O)
    wv0 = w[:, 0] - w[:, 2] - w[:, 4]
    wcat = np.concatenate(
        [16.0 * w[:, 1], 2.0 * w[:, 2] / 16.0,
         16.0 * w[:, 3], 2.0 * w[:, 4] / 16.0,
         SCALE * 16.0 * wv0], axis=1
    ).astype(NPBF16)  # [128, 320]; hop slots scaled so fp8 adds stay in-range

    def strip_img(s):
        # fp8 DoubleRow pair layout: [t, p, ktp*256 + kt2*128 + j]
        #   = fp8(SCALE * s[t*128+j, (ktp*2+kt2)*128 + p])
        r = (SCALE * np.asarray(s, np.float32)).astype(NPFP8)
        r = r.reshape(NT, 128, NT, 128).transpose(0, 3, 2, 1)  # [t, p, kt, j]
        return np.ascontiguousarray(r.reshape(NT, 128, N))

    s0i, s1i = strip_img(support0), strip_img(support1)

    in_maps = []
    for c in range(NCORES):
        shard = xs[c * BS:(c + 1) * BS]                # [8b, N, F]
        # t-major SBUF image: x0t[f, t*BS*128 + b*128 + j] = shard[b, t*128+j, f]
        x0t = np.ascontiguousarray(
            shard.reshape(BS, NT, 128, F).transpose(3, 1, 0, 2).reshape(F, BS * N)
        ).astype(NPBF16)
        in_maps.append({"x0t": x0t, "wcat": wcat, "s0t": s0i, "s1t": s1i})
    return in_maps


def postprocess(results, biases):
    full = np.empty((B, N, O), np.float32)
    for c, r in enumerate(results):
        full[c * BS:(c + 1) * BS] = (
            np.asarray(r["out"], dtype=np.float32).reshape(N, BS, O).transpose(1, 0, 2)
        )
    full += np.asarray(biases, np.float32)[None, None, :]
    return full.reshape(B, N * O)


def kernel(support0, support1, inputs, state, weight, biases, output_size=None,
           **run_kwargs):
    nc = _get_nc()
    in_maps = make_inputs(support0, support1, inputs, state, weight)
    res = run_bass_kernel_spmd(nc, in_maps, core_ids=list(range(NCORES)),
                               **run_kwargs)
    out = postprocess(res.results, biases)
    if run_kwargs.get("trace"):
        return out, res
    return out
